# revision 7
# baseline (speedup 1.0000x reference)
"""Multi-head self-attention (B=4, S=2048, E=1024, H=16) on 8 TRN2 NeuronCores.

Sharding: 4-way data parallel over batch x 2-way tensor parallel over heads
(8 heads per head-group). Core c = b*2 + g handles batch b, head-group g.
Each core computes a partial output y_partial[b,g] = attn_out_g @ out_w[:, g]^T;
the host sums the two head-group partials per batch and adds out_b.

On-chip layout strategy (all matmul operands bf16, PSUM accumulation fp32):
  - Host pre-transposes so the contraction dim always lands on SBUF partitions:
      xT  [E, S]   (e-part)    = qkv[b].T
      wT  [E, 3*EH] (e-part)   = in_w rows for this group, transposed
      owT [EH, E]  (e'-part)   = out_w cols for this group, transposed
  - in_proj emits Q^T, K^T in [f, s] layout (feature-on-partition) and V in
    natural [s, f] layout, both straight from the PE without any transposes.
  - scores^T[k, q] matmuls contract D=64; two heads are packed into the 128 PE
    rows (row-group tiling) so the array stays full.
  - softmax: no max-subtraction needed (scores ~ N(0,1)); exp runs on the ACT
    engine straight out of PSUM with scale=1/sqrt(D) folded in, writing bf16.
  - P@V uses V augmented with a ones column (M=65) so the softmax denominator
    accumulates for free in PSUM row 64.
  - normalization happens on eviction: recip (DVE) -> partition_broadcast
    (GPSIMD) -> multiply+cast into attn_out^T (DVE).
  - out_proj contracts e'=512 over 4 partition tiles, output in natural [s, f].
"""

import numpy as np
import ml_dtypes

import concourse.bass as bass
import concourse.mybir as mybir
import concourse.tile as tile
from concourse import bacc
from concourse.bass_utils import run_bass_kernel_spmd

BF16 = mybir.dt.bfloat16
F32 = mybir.dt.float32

B, S, E, H = 4, 2048, 1024, 16
D = 64
N_CORES = 8
GROUPS = 2                 # head-group tensor parallel
H_LOC = H // GROUPS        # 8 heads per core
EH = H_LOC * D             # 512 per-core head features
PAIRS = H_LOC // 2         # 4 head pairs
QC = 512                   # q chunk (scores free dim)


def build_nc(s=S, reps=1):
    """Build the per-core Bass program. `s` (sequence length) is parametrized
    so a scaled-down version can run in CoreSim. `reps` > 1 wraps the whole
    body in a hardware loop (used only by the timing harness to amortize
    dispatch overhead out of wall-clock measurements)."""
    assert s % QC == 0 and s % 128 == 0
    n_qc = s // QC           # q chunks of 512
    n_kc = s // 128          # k chunks of 128
    n_st = s // 128          # s-tiles of 128
    ET = E // 128            # 8 e-tiles (in_proj contraction)
    FT = (2 * EH) // 128     # 8 f-chunks for q+k features
    OT = EH // 128           # 4 e'-tiles (out_proj contraction)

    nc = bacc.Bacc(None, target_bir_lowering=False, debug=False)

    xT_d = nc.dram_tensor("xT", [128, ET, s], BF16, kind="ExternalInput")
    wT_d = nc.dram_tensor("wT", [128, ET, 3 * EH], BF16, kind="ExternalInput")
    bqk_d = nc.dram_tensor("bqk", [128, FT], F32, kind="ExternalInput")
    bv_d = nc.dram_tensor("bv", [128, EH], BF16, kind="ExternalInput")
    owT_d = nc.dram_tensor("owT", [128, OT, E], BF16, kind="ExternalInput")
    y_d = nc.dram_tensor("y", [s, E], F32, kind="ExternalOutput")

    with tile.TileContext(nc) as tc:
        with (
            tc.tile_pool(name="const", bufs=1) as const,
            tc.tile_pool(name="state", bufs=1) as state,
            tc.tile_pool(name="work", bufs=6) as work,
            tc.tile_pool(name="norm", bufs=3) as norm,
            # PSUM budget (8 banks): scores double-buffered 2x2, pv single
            # 1x2 (fast-evicted), in/out_proj accumulators 2x1.
            tc.tile_pool(name="ps_sc", bufs=2, space="PSUM") as ps_sc,
            tc.tile_pool(name="ps_pv", bufs=1, space="PSUM") as ps_pv,
            tc.tile_pool(name="ps_misc", bufs=2, space="PSUM") as ps_misc,
        ):
            def emit():
                # ---- load inputs (split so compute can start early) ----
                xT = const.tile([128, ET, s], BF16, name="xT_sb", tag="xT_sb")
                wT = const.tile([128, ET, 3 * EH], BF16, name="wT_sb", tag="wT_sb")
                bqk = const.tile([128, FT], F32, name="bqk_sb", tag="bqk_sb")
                bv = const.tile([128, EH], BF16, name="bv_sb", tag="bv_sb")
                owT = const.tile([128, OT, E], BF16, name="owT_sb", tag="owT_sb")
                for k in range(ET):
                    nc.sync.dma_start(wT[:, k], wT_d[:, k])
                    nc.sync.dma_start(xT[:, k], xT_d[:, k])
                nc.sync.dma_start(bqk[:], bqk_d[:])
                nc.sync.dma_start(bv[:], bv_d[:])
                nc.sync.dma_start(owT[:], owT_d[:])

                # ---- persistent intermediates ----
                # Q^T / K^T per head pair: partitions 0:64 = head 2p,
                # 64:128 = head 2p+1
                QT = [state.tile([128, s], BF16, name=f"QT{p}", tag=f"QT{p}")
                      for p in range(PAIRS)]
                KT = [state.tile([128, s], BF16, name=f"KT{p}", tag=f"KT{p}")
                      for p in range(PAIRS)]
                # V natural [s, f] with ones column per head:
                # [128, st, h*65+d], col 64 = 1
                V = state.tile([128, n_st, H_LOC * (D + 1)], BF16,
                               name="V_sb", tag="V_sb")
                # attn_out^T, e' on partitions: tile p holds heads (2p, 2p+1)
                aoT = state.tile([128, OT, s], BF16, name="aoT", tag="aoT")

                nc.vector.memset(V[:, :, D::D + 1], 1.0)  # ones columns

                # ---- in_proj: V (natural layout) ----
                # V[s,f] = x @ Wv^T : lhsT = xT (M = s-chunk), rhs = wvT
                for m in range(n_st):
                    ps = ps_misc.tile([128, EH], F32, name="ps_v", tag="ps_misc")
                    for k in range(ET):
                        nc.tensor.matmul(
                            ps[:],
                            xT[:, k, m * 128:(m + 1) * 128],
                            wT[:, k, 2 * EH:3 * EH],
                            start=(k == 0), stop=(k == ET - 1),
                        )
                    # evict + bias (broadcast along partitions via replicated tile)
                    nc.vector.tensor_tensor(
                        V[:, m, :].rearrange("p (h d) -> p h d", h=H_LOC)[:, :, 0:D],
                        ps[:].rearrange("p (h d) -> p h d", h=H_LOC),
                        bv[:].rearrange("p (h d) -> p h d", h=H_LOC),
                        mybir.AluOpType.add,
                    )

                # ---- per head pair: in_proj QK then attention ----
                for p in range(PAIRS):
                    # Q features f-chunk p, K features f-chunk PAIRS+p
                    for which, dst in ((p, QT[p]), (PAIRS + p, KT[p])):
                        for n in range(n_qc):
                            ps = ps_misc.tile([128, QC], F32, name="ps_qk",
                                              tag="ps_misc")
                            for k in range(ET):
                                nc.tensor.matmul(
                                    ps[:],
                                    wT[:, k, which * 128:(which + 1) * 128],
                                    xT[:, k, n * QC:(n + 1) * QC],
                                    start=(k == 0), stop=(k == ET - 1),
                                )
                            nc.vector.tensor_scalar_add(
                                dst[:, n * QC:(n + 1) * QC], ps[:],
                                bqk[:, which:which + 1],
                            )

                    # attention for heads (2p, 2p+1)
                    for qc in range(n_qc):
                        pv = ps_pv.tile([128, 2, QC], F32, name="pv", tag="pv")
                        for kc in range(n_kc):
                            sc = ps_sc.tile([128, 2, QC], F32, name="sc", tag="ps")
                            ex = work.tile([128, 2, QC], BF16, name="ex", tag="ex")
                            for h01 in range(2):
                                nc.tensor.matmul(
                                    sc[:, h01, :],
                                    KT[p][h01 * D:(h01 + 1) * D,
                                          kc * 128:(kc + 1) * 128],
                                    QT[p][h01 * D:(h01 + 1) * D,
                                          qc * QC:(qc + 1) * QC],
                                    start=True, stop=True,
                                )
                            # exp((Q K^T)/sqrt(D)) from PSUM, cast to bf16
                            nc.scalar.activation(
                                ex[:], sc[:],
                                mybir.ActivationFunctionType.Exp,
                                scale=1.0 / float(np.sqrt(D)),
                            )
                            for h01 in range(2):
                                h = 2 * p + h01
                                nc.tensor.matmul(
                                    pv[0:D + 1, h01, :],
                                    V[:, kc, h * (D + 1):(h + 1) * (D + 1)],
                                    ex[:, h01, :],
                                    start=(kc == 0), stop=(kc == n_kc - 1),
                                )
                        # fast eviction frees the pv slot; normalization
                        # runs decoupled from the attention pipeline
                        pvraw = norm.tile([D + 1, 2, QC], F32, name="pvraw",
                                          tag="pvraw")
                        nc.vector.tensor_copy(pvraw[:], pv[0:D + 1])
                        for h01 in range(2):
                            r = norm.tile([1, QC], F32, name="r", tag="r")
                            rr = norm.tile([D, QC], F32, name="rr", tag="rr")
                            nc.vector.reciprocal(r[:], pvraw[D:D + 1, h01, :])
                            nc.gpsimd.partition_broadcast(rr[:], r[:])
                            nc.vector.tensor_tensor(
                                aoT[h01 * D:(h01 + 1) * D, p,
                                    qc * QC:(qc + 1) * QC],
                                pvraw[0:D, h01, :],
                                rr[:],
                                mybir.AluOpType.mult,
                            )

                # ---- out_proj: y[s, f] partial ----
                for m in range(n_st):
                    for fc in range(E // 512):
                        ps = ps_misc.tile([128, 512], F32, name="ps_o",
                                          tag="ps_misc")
                        for t in range(OT):
                            nc.tensor.matmul(
                                ps[:],
                                aoT[:, t, m * 128:(m + 1) * 128],
                                owT[:, t, fc * 512:(fc + 1) * 512],
                                start=(t == 0), stop=(t == OT - 1),
                            )
                        ysb = work.tile([128, 512], F32, name="ysb", tag="ysb")
                        nc.vector.tensor_copy(ysb[:], ps[:])
                        nc.sync.dma_start(
                            y_d[m * 128:(m + 1) * 128, fc * 512:(fc + 1) * 512],
                            ysb[:],
                        )

            if reps == 1:
                emit()
            else:
                with tc.For_i(0, reps, 1, hint_engines=(
                        mybir.EngineType.PE, mybir.EngineType.Activation,
                        mybir.EngineType.DVE, mybir.EngineType.SP,
                        mybir.EngineType.Pool)):
                    emit()

    nc.compile()
    return nc


def shard_inputs(qkv, in_w, in_b, out_w, s=S):
    """Host-side shard + transpose + cast. Returns in_maps for the 8 cores."""
    ET = E // 128
    FT = (2 * EH) // 128
    OT = EH // 128
    bf16 = ml_dtypes.bfloat16
    in_maps = []
    for c in range(N_CORES):
        b, g = divmod(c, GROUPS)
        x = np.asarray(qkv[b], dtype=np.float32)              # [s, E]
        xT = np.ascontiguousarray(
            x.T.reshape(ET, 128, s).transpose(1, 0, 2)).astype(bf16)
        rows = np.concatenate([
            in_w[g * EH:(g + 1) * EH],
            in_w[E + g * EH:E + (g + 1) * EH],
            in_w[2 * E + g * EH:2 * E + (g + 1) * EH],
        ], axis=0)                                            # [3*EH, E]
        wT = np.ascontiguousarray(
            rows.T.reshape(ET, 128, 3 * EH).transpose(1, 0, 2)).astype(bf16)
        bqk = np.concatenate([
            in_b[g * EH:(g + 1) * EH], in_b[E + g * EH:E + (g + 1) * EH]
        ]).reshape(FT, 128).T.astype(np.float32)
        bqk = np.ascontiguousarray(bqk)
        bv = np.ascontiguousarray(np.broadcast_to(
            in_b[2 * E + g * EH:2 * E + (g + 1) * EH].astype(bf16), (128, EH)))
        ow = out_w[:, g * EH:(g + 1) * EH]                    # [E, EH]
        owT = np.ascontiguousarray(
            ow.T.reshape(OT, 128, E).transpose(1, 0, 2)).astype(bf16)
        in_maps.append({"xT": xT, "wT": wT, "bqk": bqk, "bv": bv, "owT": owT})
    return in_maps


_NC_CACHE = {}


def kernel(qkv, in_w, in_b, out_w, out_b):
    qkv = np.asarray(qkv, np.float32)
    in_w = np.asarray(in_w, np.float32)
    in_b = np.asarray(in_b, np.float32)
    out_w = np.asarray(out_w, np.float32)
    out_b = np.asarray(out_b, np.float32)

    if S not in _NC_CACHE:
        _NC_CACHE[S] = build_nc(S)
    nc = _NC_CACHE[S]

    in_maps = shard_inputs(qkv, in_w, in_b, out_w)
    res = run_bass_kernel_spmd(nc, in_maps, core_ids=list(range(N_CORES)))
    out = np.empty((B, S, E), np.float32)
    for b in range(B):
        out[b] = res.results[b * GROUPS + 0]["y"] + res.results[b * GROUPS + 1]["y"] \
            + out_b[None, :]
    return out


# revision 11
# speedup vs baseline: 8.1071x; 8.1071x over previous
"""Multi-head self-attention (B=4, S=2048, E=1024, H=16) on 8 TRN2 NeuronCores.

Sharding: 4-way data parallel over batch x 2-way tensor parallel over heads
(8 heads per head-group). Core c = b*2 + g handles batch b, head-group g.
Each core computes a partial output y_partial[b,g] = attn_out_g @ out_w[:, g]^T;
the host sums the two head-group partials per batch and adds out_b.

On-chip layout strategy (all matmul operands bf16, PSUM accumulation fp32):
  - Host pre-transposes so the contraction dim always lands on SBUF partitions:
      xT  [E, S]   (e-part)    = qkv[b].T
      wT  [E, 3*EH] (e-part)   = in_w rows for this group, transposed
      owT [EH, E]  (e'-part)   = out_w cols for this group, transposed
  - in_proj emits Q^T, K^T in [f, s] layout (feature-on-partition) and V in
    natural [s, f] layout, both straight from the PE without any transposes.
  - scores^T[k, q] matmuls contract D=64; two heads are packed into the 128 PE
    rows (row-group tiling, tile_position auto-derived from base_partition) so
    the array stays full.
  - softmax: no max-subtraction needed (scores ~ N(0,1), exp range is tiny);
    exp runs on the ACT engine straight out of PSUM with scale=1/sqrt(D)
    folded into the activation's free affine, writing bf16.
  - P@V uses V augmented with a ones column (M=65) so the softmax denominator
    accumulates for free in PSUM row 64.
  - PV PSUM is evicted with one fast DVE copy (frees the bank quickly); the
    normalization (recip -> gpsimd partition_broadcast -> multiply) runs
    decoupled from the attention pipeline.
  - out_proj contracts e'=512 over 4 partition tiles, output in natural [s, f].
"""

import numpy as np
import ml_dtypes

import concourse.bass as bass
import concourse.mybir as mybir
import concourse.tile as tile
from concourse import bacc
from concourse.bass_utils import run_bass_kernel_spmd

BF16 = mybir.dt.bfloat16
F32 = mybir.dt.float32

B, S, E, H = 4, 2048, 1024, 16
D = 64
N_CORES = 8
GROUPS = 2                 # head-group tensor parallel
H_LOC = H // GROUPS        # 8 heads per core
EH = H_LOC * D             # 512 per-core head features
PAIRS = H_LOC // 2         # 4 head pairs
QC = 512                   # q chunk (scores free dim)


def build_nc(s=S, reps=1):
    """Build the per-core Bass program. `s` (sequence length) is parametrized
    so a scaled-down version can run in CoreSim. `reps` > 1 wraps the whole
    body in a hardware loop (used only by the timing harness to amortize
    dispatch overhead out of wall-clock measurements)."""
    assert s % QC == 0 and s % 128 == 0
    n_qc = s // QC           # q chunks of 512
    n_kc = s // 128          # k chunks of 128
    n_st = s // 128          # s-tiles of 128
    ET = E // 128            # 8 e-tiles (in_proj contraction)
    FT = (2 * EH) // 128     # 8 f-chunks for q+k features
    OT = EH // 128           # 4 e'-tiles (out_proj contraction)

    nc = bacc.Bacc(None, target_bir_lowering=False, debug=False)

    xT_d = nc.dram_tensor("xT", [128, ET, s], BF16, kind="ExternalInput")
    wT_d = nc.dram_tensor("wT", [128, ET, 3 * EH], BF16, kind="ExternalInput")
    bqk_d = nc.dram_tensor("bqk", [128, FT], F32, kind="ExternalInput")
    bv_d = nc.dram_tensor("bv", [128, EH], BF16, kind="ExternalInput")
    owT_d = nc.dram_tensor("owT", [128, OT, E], BF16, kind="ExternalInput")
    y_d = nc.dram_tensor("y", [s, E], F32, kind="ExternalOutput")

    with tile.TileContext(nc) as tc:
        with (
            tc.tile_pool(name="const", bufs=1) as const,
            tc.tile_pool(name="state", bufs=1) as state,
            tc.tile_pool(name="work", bufs=6) as work,
            tc.tile_pool(name="norm", bufs=3) as norm,
            # PSUM budget (8 banks): scores double-buffered 2x2, pv single
            # 1x2 (fast-evicted), in/out_proj accumulators 2x1.
            tc.tile_pool(name="ps_sc", bufs=2, space="PSUM") as ps_sc,
            tc.tile_pool(name="ps_pv", bufs=1, space="PSUM") as ps_pv,
            tc.tile_pool(name="ps_misc", bufs=2, space="PSUM") as ps_misc,
        ):
            def emit():
                # ---- load inputs (split so compute can start early) ----
                xT = const.tile([128, ET, s], BF16, name="xT_sb", tag="xT_sb")
                wT = const.tile([128, ET, 3 * EH], BF16, name="wT_sb", tag="wT_sb")
                bqk = const.tile([128, FT], F32, name="bqk_sb", tag="bqk_sb")
                bv = const.tile([128, EH], BF16, name="bv_sb", tag="bv_sb")
                owT = const.tile([128, OT, E], BF16, name="owT_sb", tag="owT_sb")
                for k in range(ET):
                    nc.sync.dma_start(wT[:, k], wT_d[:, k])
                    nc.sync.dma_start(xT[:, k], xT_d[:, k])
                nc.sync.dma_start(bqk[:], bqk_d[:])
                nc.sync.dma_start(bv[:], bv_d[:])
                nc.sync.dma_start(owT[:], owT_d[:])

                # ---- persistent intermediates ----
                # Q^T / K^T per head pair: partitions 0:64 = head 2p,
                # 64:128 = head 2p+1
                QT = [state.tile([128, s], BF16, name=f"QT{p}", tag=f"QT{p}")
                      for p in range(PAIRS)]
                KT = [state.tile([128, s], BF16, name=f"KT{p}", tag=f"KT{p}")
                      for p in range(PAIRS)]
                # V natural [s, f] with ones column per head:
                # [128, st, h*65+d], col 64 = 1
                V = state.tile([128, n_st, H_LOC * (D + 1)], BF16,
                               name="V_sb", tag="V_sb")
                # attn_out^T, e' on partitions: tile p holds heads (2p, 2p+1)
                aoT = state.tile([128, OT, s], BF16, name="aoT", tag="aoT")

                nc.vector.memset(V[:, :, D::D + 1], 1.0)  # ones columns

                # ---- in_proj: V (natural layout) ----
                # V[s,f] = x @ Wv^T : lhsT = xT (M = s-chunk), rhs = wvT
                for m in range(n_st):
                    ps = ps_misc.tile([128, EH], F32, name="ps_v", tag="ps_misc")
                    for k in range(ET):
                        nc.tensor.matmul(
                            ps[:],
                            xT[:, k, m * 128:(m + 1) * 128],
                            wT[:, k, 2 * EH:3 * EH],
                            start=(k == 0), stop=(k == ET - 1),
                        )
                    # evict + bias (broadcast along partitions via replicated
                    # tile prepared on the host)
                    nc.vector.tensor_tensor(
                        V[:, m, :].rearrange("p (h d) -> p h d", h=H_LOC)[:, :, 0:D],
                        ps[:].rearrange("p (h d) -> p h d", h=H_LOC),
                        bv[:].rearrange("p (h d) -> p h d", h=H_LOC),
                        mybir.AluOpType.add,
                    )

                # ---- per head pair: in_proj QK then attention ----
                for p in range(PAIRS):
                    # Q features f-chunk p, K features f-chunk PAIRS+p
                    for which, dst in ((p, QT[p]), (PAIRS + p, KT[p])):
                        for n in range(n_qc):
                            ps = ps_misc.tile([128, QC], F32, name="ps_qk",
                                              tag="ps_misc")
                            for k in range(ET):
                                nc.tensor.matmul(
                                    ps[:],
                                    wT[:, k, which * 128:(which + 1) * 128],
                                    xT[:, k, n * QC:(n + 1) * QC],
                                    start=(k == 0), stop=(k == ET - 1),
                                )
                            nc.vector.tensor_scalar_add(
                                dst[:, n * QC:(n + 1) * QC], ps[:],
                                bqk[:, which:which + 1],
                            )

                    # attention for heads (2p, 2p+1)
                    for qc in range(n_qc):
                        pv = ps_pv.tile([128, 2, QC], F32, name="pv", tag="pv")
                        for kc in range(n_kc):
                            sc = ps_sc.tile([128, 2, QC], F32, name="sc",
                                            tag="sc")
                            ex = work.tile([128, 2, QC], BF16, name="ex",
                                           tag="ex")
                            for h01 in range(2):
                                nc.tensor.matmul(
                                    sc[:, h01, :],
                                    KT[p][h01 * D:(h01 + 1) * D,
                                          kc * 128:(kc + 1) * 128],
                                    QT[p][h01 * D:(h01 + 1) * D,
                                          qc * QC:(qc + 1) * QC],
                                    start=True, stop=True,
                                )
                            # exp((Q K^T)/sqrt(D)) from PSUM, cast to bf16
                            nc.scalar.activation(
                                ex[:], sc[:],
                                mybir.ActivationFunctionType.Exp,
                                scale=1.0 / float(np.sqrt(D)),
                            )
                            for h01 in range(2):
                                h = 2 * p + h01
                                nc.tensor.matmul(
                                    pv[0:D + 1, h01, :],
                                    V[:, kc, h * (D + 1):(h + 1) * (D + 1)],
                                    ex[:, h01, :],
                                    start=(kc == 0), stop=(kc == n_kc - 1),
                                )
                        # fast eviction frees the pv slot; normalization
                        # runs decoupled from the attention pipeline
                        pvraw = norm.tile([D + 1, 2, QC], F32, name="pvraw",
                                          tag="pvraw")
                        nc.vector.tensor_copy(pvraw[:], pv[0:D + 1])
                        for h01 in range(2):
                            r = norm.tile([1, QC], F32, name="r", tag="r")
                            rr = norm.tile([D, QC], F32, name="rr", tag="rr")
                            nc.vector.reciprocal(r[:], pvraw[D:D + 1, h01, :])
                            nc.gpsimd.partition_broadcast(rr[:], r[:])
                            nc.vector.tensor_tensor(
                                aoT[h01 * D:(h01 + 1) * D, p,
                                    qc * QC:(qc + 1) * QC],
                                pvraw[0:D, h01, :],
                                rr[:],
                                mybir.AluOpType.mult,
                            )

                # ---- out_proj: y[s, f] partial ----
                for m in range(n_st):
                    for fc in range(E // 512):
                        ps = ps_misc.tile([128, 512], F32, name="ps_o",
                                          tag="ps_misc")
                        for t in range(OT):
                            nc.tensor.matmul(
                                ps[:],
                                aoT[:, t, m * 128:(m + 1) * 128],
                                owT[:, t, fc * 512:(fc + 1) * 512],
                                start=(t == 0), stop=(t == OT - 1),
                            )
                        ysb = work.tile([128, 512], F32, name="ysb", tag="ysb")
                        nc.vector.tensor_copy(ysb[:], ps[:])
                        nc.sync.dma_start(
                            y_d[m * 128:(m + 1) * 128, fc * 512:(fc + 1) * 512],
                            ysb[:],
                        )

            if reps == 1:
                emit()
            else:
                with tc.For_i(0, reps, 1, hint_engines=(
                        mybir.EngineType.PE, mybir.EngineType.Activation,
                        mybir.EngineType.DVE, mybir.EngineType.SP,
                        mybir.EngineType.Pool)):
                    emit()

    nc.compile()
    return nc


def shard_inputs(qkv, in_w, in_b, out_w, s=S):
    """Host-side shard + transpose + cast. Returns in_maps for the 8 cores."""
    ET = E // 128
    FT = (2 * EH) // 128
    OT = EH // 128
    bf16 = ml_dtypes.bfloat16
    in_maps = []
    for c in range(N_CORES):
        b, g = divmod(c, GROUPS)
        x = np.asarray(qkv[b], dtype=np.float32)              # [s, E]
        xT = np.ascontiguousarray(
            x.T.reshape(ET, 128, s).transpose(1, 0, 2)).astype(bf16)
        rows = np.concatenate([
            in_w[g * EH:(g + 1) * EH],
            in_w[E + g * EH:E + (g + 1) * EH],
            in_w[2 * E + g * EH:2 * E + (g + 1) * EH],
        ], axis=0)                                            # [3*EH, E]
        wT = np.ascontiguousarray(
            rows.T.reshape(ET, 128, 3 * EH).transpose(1, 0, 2)).astype(bf16)
        bqk = np.concatenate([
            in_b[g * EH:(g + 1) * EH], in_b[E + g * EH:E + (g + 1) * EH]
        ]).reshape(FT, 128).T.astype(np.float32)
        bqk = np.ascontiguousarray(bqk)
        bv = np.ascontiguousarray(np.broadcast_to(
            in_b[2 * E + g * EH:2 * E + (g + 1) * EH].astype(bf16), (128, EH)))
        ow = out_w[:, g * EH:(g + 1) * EH]                    # [E, EH]
        owT = np.ascontiguousarray(
            ow.T.reshape(OT, 128, E).transpose(1, 0, 2)).astype(bf16)
        in_maps.append({"xT": xT, "wT": wT, "bqk": bqk, "bv": bv, "owT": owT})
    return in_maps


_NC_CACHE = {}


def kernel(qkv, in_w, in_b, out_w, out_b):
    qkv = np.asarray(qkv, np.float32)
    in_w = np.asarray(in_w, np.float32)
    in_b = np.asarray(in_b, np.float32)
    out_w = np.asarray(out_w, np.float32)
    out_b = np.asarray(out_b, np.float32)

    if S not in _NC_CACHE:
        _NC_CACHE[S] = build_nc(S)
    nc = _NC_CACHE[S]

    in_maps = shard_inputs(qkv, in_w, in_b, out_w)
    res = run_bass_kernel_spmd(nc, in_maps, core_ids=list(range(N_CORES)))
    out = np.empty((B, S, E), np.float32)
    for b in range(B):
        out[b] = res.results[b * GROUPS + 0]["y"] + res.results[b * GROUPS + 1]["y"] \
            + out_b[None, :]
    return out
